# revision 12
# baseline (speedup 1.0000x reference)
"""Trainium2 Bass kernel for ColorHistogramLoss.

Reference computation:
  brightness = mean(target, axis=1)           # [B,1,H,W]
  mask = brightness > 0.4
  soft 16-bin Gaussian histograms of pred/target per (b, c), masked,
  normalized; loss = mean |pred_hist - target_hist|.

Kernel strategy (8 NeuronCores, data-parallel over batch B=8), v2:
  Each core processes one image pair (pred[b], target[b]) [3,512,512].

  Instead of evaluating 16 exps per element (ScalarE-bound at ~178us)
  or 16 DVE scalar_tensor_tensor ops per element (DVE-bound at ~225us,
  the v1 baseline), exploit the multiplicative structure of the
  Gaussian row: with w_k = exp(-128(x - k/15)^2),

      w_{k+1} = w_k * r_up * qup_k,   r_up = exp((256/15)x - b)
      w_{k-1} = w_k * r_dn * qdn_k,   r_dn = exp(-(256/15)x - b)

  (qup/qdn are per-step compile-time scalars).  So the whole 16-bin row
  costs 2 ScalarE exps (r_up/r_dn) + 1 fp16 DVE stt per derived bin.
  fp16 (not bf16) is needed for precision (chain error compounds), but
  fp16's narrow range underflows mid-chain, so the row is split into
  two 8-bin chains, each freshly started at its center bin (4 and 12)
  with w_start = exp(-128*Square(xm - c) + 13*ln2) (ScalarE Square+Exp)
  and run bidirectionally.  The 2^13 start scale + e^{-+6} r-tile
  rescales keep every stored fp16 value in [6e-8, 65504]; host divides
  the scale back out.  Verified numerically: loss rel err ~1e-4.

  The brightness mask folds in as xm = x + 100*(1-m): masked elements
  sit ~100 away from every bin center, so every chain start underflows
  to exactly 0 and the recurrence keeps them at 0 in all bins.

  Per-bin accumulation runs on the otherwise-idle TensorEngine: a
  [128,2] ones stationary (one column per stacked channel half) sums
  each w_k tile into PSUM rows (32*pair + 2*bin + half) via 8
  accumulating 512-column matmuls.  One final DVE tensor_reduce
  collapses PSUM [96,512] -> [96,1] which is DMA'd out; the tiny
  normalize / L1 / mean finish runs on host.

  Layout: channels PAIR-STACKED on the partition axis as in v1: a
  [128, 4096] tile holds channel A on partitions 0..63, B on 64..127.

  Predicted engine busy per core: DVE ~118us (bound), PE ~83us,
  ScalarE ~67us, vs v1's DVE ~225us.
"""

from contextlib import ExitStack
import math

import numpy as np

import concourse.bass as bass
import concourse.tile as tile
from concourse import bacc, mybir
from concourse.bass_utils import run_bass_kernel_spmd

N_CORES = 8
C = 3
H = 512
W = 512
HW = H * W          # 262144
P = 128
HP = 64             # partitions per channel in a stacked pair
FP = HW // HP       # 4096
NB = 16
NPAIR = 3           # (t1,t2), (p2,t0), (p0,p1)
F32 = mybir.dt.float32
F16 = mybir.dt.float16

MASK_ON_GPS = False           # brightness sum + threshold on GPSIMD
XM_ON_GPS = False             # xm = x + off2 on GPSIMD
BETA = 128.0 / 225.0          # exp(-128(x-k/15)^2) = exp(-BETA (15x-k)^2)
A = 256.0 / 15.0              # d/dx of the up-ratio exponent
RS = 6.0                      # r-tile rescale (fp16 range)
SC = 15.0 * math.log(2.0)     # chain-start scale 2^15
CHAIN_STARTS = (0, 4, 8, 12)  # ascending-only chains of 4 bins each
CHAIN_STEPS = 3
MM_CHUNK = 512                # matmul moving free-dim (PSUM bank = 512 f32)
STATS_ROWS = 2 * NPAIR * NB   # 96


def _kernel_body(ctx, tc, stats_d, pred_d, target_d, repeat=1):
    nc = tc.nc
    stacks = ctx.enter_context(tc.tile_pool(name="stacks", bufs=1))
    maskp = ctx.enter_context(tc.tile_pool(name="maskp", bufs=1))
    scr = ctx.enter_context(tc.tile_pool(name="scr", bufs=1))
    vpool = ctx.enter_context(tc.tile_pool(name="vpool", bufs=1))
    epool = ctx.enter_context(tc.tile_pool(name="epool", bufs=10))
    ppool = ctx.enter_context(tc.tile_pool(name="ppool", bufs=1, space="PSUM"))
    spool = ctx.enter_context(tc.tile_pool(name="spool", bufs=1))
    pools = (stacks, maskp, scr, vpool, epool, ppool, spool)

    # Per-bin [128, 32] fp16 stationaries: bin k has ones at
    # (partitions 0..63, col 2k) and (64..127, col 2k+1), zeros elsewhere.
    # Matmul out [32, 512] lands at PSUM base 32*pair (base must be
    # 0/32/64); the zero columns contribute 0 to the other bins' rows.
    ones_k = []
    for k in range(NB):
        o = spool.tile([P, 2 * NB], F16, tag=f"ones{k}")
        nc.gpsimd.memset(o[:], 0.0)
        nc.gpsimd.memset(o[:HP, 2 * k : 2 * k + 1], 1.0)
        nc.gpsimd.memset(o[HP:, 2 * k + 1 : 2 * k + 2], 1.0)
        ones_k.append(o)

    # ACT bias constants must be APs: one [128, 1] column per value.
    bias_vals = [-BETA - RS, SC] + [-s0 / 15.0 for s0 in CHAIN_STARTS]
    biases = spool.tile([P, len(bias_vals)], F32, tag="biases")
    bias_ap = {}
    for i, v in enumerate(bias_vals):
        nc.gpsimd.memset(biases[:, i : i + 1], v)
        bias_ap[v] = biases[:, i : i + 1]

    for _ in range(repeat):
        _emit_pass(ctx, tc, pools, ones_k, bias_ap, stats_d, pred_d, target_d)


def _emit_pass(ctx, tc, pools, ones_k, bias_ap, stats_d, pred_d, target_d):
    nc = tc.nc
    add = mybir.AluOpType.add
    mult = mybir.AluOpType.mult
    is_le = mybir.AluOpType.is_le
    stacks, maskp, scr, vpool, epool, ppool, spool = pools

    def chan_ap(dram, c):
        return dram[c].rearrange("(q g) -> q g", q=HP)

    pair_srcs = [
        (chan_ap(target_d, 1), chan_ap(target_d, 2)),
        (chan_ap(pred_d, 2), chan_ap(target_d, 0)),
        (chan_ap(pred_d, 0), chan_ap(pred_d, 1)),
    ]
    # pair0 (mask inputs) first, plus base-0 re-reads of t0/t2 so the
    # brightness sum has all three channels at the same base partition.
    pair_tiles = []
    t = stacks.tile([P, FP], F32, tag="pair0")
    nc.sync.dma_start(out=t[:HP, :], in_=pair_srcs[0][0])
    nc.sync.dma_start(out=t[HP:, :], in_=pair_srcs[0][1])
    pair_tiles.append(t)
    m0 = scr.tile([P, FP], F32, tag="m0")
    nc.sync.dma_start(out=m0[:HP, :], in_=chan_ap(target_d, 0))
    m2 = scr.tile([P, FP], F32, tag="m2")
    nc.sync.dma_start(out=m2[:HP, :], in_=chan_ap(target_d, 2))
    for i, (a_ap, b_ap) in enumerate(pair_srcs[1:], start=1):
        t = stacks.tile([P, FP], F32, tag=f"pair{i}")
        nc.sync.dma_start(out=t[:HP, :], in_=a_ap)
        nc.sync.dma_start(out=t[HP:, :], in_=b_ap)
        pair_tiles.append(t)

    # off2 = 100 where masked out (brightness sum <= 1.2), else 0.
    off2 = maskp.tile([P, FP], F32, tag="off2")
    s = off2[:HP, :]
    meng = nc.gpsimd if MASK_ON_GPS else nc.vector
    meng.tensor_tensor(out=s, in0=m0[:HP, :], in1=pair_tiles[0][:HP, :], op=add)
    meng.tensor_tensor(out=s, in0=s, in1=m2[:HP, :], op=add)
    meng.tensor_scalar(
        out=s, in0=s, scalar1=1.2, scalar2=100.0, op0=is_le, op1=mult
    )
    nc.sync.dma_start(out=off2[HP:, :], in_=s)  # replicate to upper half

    psum = ppool.tile([STATS_ROWS, MM_CHUNK], F32, tag="psum")
    nchunk = FP // MM_CHUNK
    # per pair-region: 16 bins x nchunk accumulating matmuls; first resets,
    # last closes the group.
    mm_total = NB * nchunk
    mm_count = [0, 0, 0]

    def mm(w, pi, k):
        base = 32 * pi
        for c in range(nchunk):
            n = mm_count[pi]
            nc.tensor.matmul(
                out=psum[base : base + 2 * NB, :],
                lhsT=ones_k[k][:],
                rhs=w[:, c * MM_CHUNK : (c + 1) * MM_CHUNK],
                start=(n == 0),
                stop=(n == mm_total - 1),
            )
            mm_count[pi] = n + 1

    # Per pair: xm + r_up prep, then the 4 ascending chains interleaved
    # step-by-step (4 independent stt sequences) so consecutive DVE ops are
    # never data-dependent (hides the DVE pipe DRAIN).  Pairs stay
    # sequential so each pair's PSUM accumulation group closes before the
    # next opens.
    for pi, x in enumerate(pair_tiles):
        xm = vpool.tile([P, FP], F16, tag="xm")
        xeng = nc.gpsimd if XM_ON_GPS else nc.vector
        xeng.tensor_tensor(out=xm[:], in0=x[:], in1=off2[:], op=add)
        r_up = vpool.tile([P, FP], F16, tag="r_up")
        nc.scalar.activation(
            out=r_up[:], in_=x[:], func=mybir.ActivationFunctionType.Exp,
            bias=bias_ap[-BETA - RS], scale=A,
        )
        wp = []
        for ci, s0 in enumerate(CHAIN_STARTS):
            sq = epool.tile([P, FP], F16, tag="e")
            nc.scalar.activation(
                out=sq[:], in_=xm[:], func=mybir.ActivationFunctionType.Square,
                bias=bias_ap[-s0 / 15.0], scale=1.0,
            )
            w0 = epool.tile([P, FP], F16, tag="e")
            nc.scalar.activation(
                out=w0[:], in_=sq[:], func=mybir.ActivationFunctionType.Exp,
                bias=bias_ap[SC], scale=-128.0,
            )
            mm(w0, pi, s0)
            wp.append(w0)
        for step in range(CHAIN_STEPS):
            for ci, s0 in enumerate(CHAIN_STARTS):
                k = s0 + step
                wn = epool.tile([P, FP], F16, tag="e")
                nc.vector.scalar_tensor_tensor(
                    out=wn[:], in0=r_up[:],
                    scalar=math.exp(-2.0 * BETA * k + RS),
                    in1=wp[ci][:], op0=mult, op1=mult,
                )
                mm(wn, pi, k + 1)
                wp[ci] = wn

    stats_t = spool.tile([STATS_ROWS, 1], F32, tag="stats")
    nc.vector.tensor_reduce(
        out=stats_t[:], in_=psum[:], axis=mybir.AxisListType.X,
        op=mybir.AluOpType.add,
    )
    nc.sync.dma_start(out=stats_d[:], in_=stats_t[:])


def build_nc(repeat=1):
    nc = bacc.Bacc(
        "TRN2", target_bir_lowering=False, debug=False, num_devices=N_CORES
    )
    pred = nc.dram_tensor("pred", [C, HW], F32, kind="ExternalInput").ap()
    target = nc.dram_tensor("target", [C, HW], F32, kind="ExternalInput").ap()
    stats = nc.dram_tensor(
        "stats", [STATS_ROWS, 1], F32, kind="ExternalOutput"
    ).ap()
    with tile.TileContext(nc) as tc:
        with ExitStack() as ctx:
            _kernel_body(ctx, tc, stats, pred, target, repeat=repeat)
    nc.compile()
    return nc


_NC_CACHE = {}


def _get_nc():
    if "nc" not in _NC_CACHE:
        _NC_CACHE["nc"] = build_nc()
    return _NC_CACHE["nc"]


# stats row -> (which hist 0=pred/1=target, channel): row = 32*pair + 2*bin + half
_PAIR_CHANNELS = [((1, 1), (1, 2)), ((0, 2), (1, 0)), ((0, 0), (0, 1))]


def stats_to_hists(stats):
    """[96, 1] per-core sums -> hist [2, C, NB] (pred, target) f64."""
    v = stats.reshape(NPAIR, NB, 2).astype(np.float64)
    hist = np.empty((2, C, NB), np.float64)
    for p in range(NPAIR):
        for half in range(2):
            which, ch = _PAIR_CHANNELS[p][half]
            hist[which, ch] = v[p, :, half]
    return hist


def finish_on_host(stats_list):
    diffs = []
    for stats in stats_list:
        hist = stats_to_hists(stats)
        hist_n = hist / (hist.sum(axis=-1, keepdims=True) + 1e-7)
        diffs.append(np.abs(hist_n[0] - hist_n[1]))
    return np.array(np.mean(np.stack(diffs)), dtype=np.float32)


def run(pred, target, **spmd_kwargs):
    nc = _get_nc()
    pred = np.ascontiguousarray(np.asarray(pred, dtype=np.float32))
    target = np.ascontiguousarray(np.asarray(target, dtype=np.float32))
    assert pred.shape == (N_CORES, C, H, W), pred.shape
    in_maps = [
        {
            "pred": pred[b].reshape(C, HW),
            "target": target[b].reshape(C, HW),
        }
        for b in range(N_CORES)
    ]
    res = run_bass_kernel_spmd(
        nc, in_maps, core_ids=list(range(N_CORES)), **spmd_kwargs
    )
    loss = finish_on_host([res.results[b]["stats"] for b in range(N_CORES)])
    return loss, res


def kernel(pred, target):
    loss, _ = run(pred, target)
    return loss


# revision 15
# speedup vs baseline: 1.9138x; 1.9138x over previous
"""Trainium2 Bass kernel for ColorHistogramLoss.

Reference computation:
  brightness = mean(target, axis=1)           # [B,1,H,W]
  mask = brightness > 0.4
  soft 16-bin Gaussian histograms of pred/target per (b, c), masked,
  normalized; loss = mean |pred_hist - target_hist|.

Kernel strategy (8 NeuronCores, data-parallel over batch B=8), v2:
  Each core processes one image pair (pred[b], target[b]) [3,512,512].

  Instead of evaluating 16 exps per element (ScalarE-bound at ~178us)
  or 16 DVE scalar_tensor_tensor ops per element (DVE-bound at ~225us,
  the v1 baseline), exploit the multiplicative structure of the
  Gaussian row: with w_k = exp(-128(x - k/15)^2),

      w_{k+1} = w_k * r_up * qup_k,   r_up = exp((256/15)x - b)
      w_{k-1} = w_k * r_dn * qdn_k,   r_dn = exp(-(256/15)x - b)

  (qup/qdn are per-step compile-time scalars).  So the whole 16-bin row
  costs 2 ScalarE exps (r_up/r_dn) + 1 fp16 DVE stt per derived bin.
  fp16 (not bf16) is needed for precision (chain error compounds), but
  fp16's narrow range underflows mid-chain, so the row is split into
  two 8-bin chains, each freshly started at its center bin (4 and 12)
  with w_start = exp(-128*Square(xm - c) + 13*ln2) (ScalarE Square+Exp)
  and run bidirectionally.  The 2^13 start scale + e^{-+6} r-tile
  rescales keep every stored fp16 value in [6e-8, 65504]; host divides
  the scale back out.  Verified numerically: loss rel err ~1e-4.

  The brightness mask folds in as xm = x + 100*(1-m): masked elements
  sit ~100 away from every bin center, so every chain start underflows
  to exactly 0 and the recurrence keeps them at 0 in all bins.

  Per-bin accumulation runs on the otherwise-idle TensorEngine: a
  [128,2] ones stationary (one column per stacked channel half) sums
  each w_k tile into PSUM rows (32*pair + 2*bin + half) via 8
  accumulating 512-column matmuls.  One final DVE tensor_reduce
  collapses PSUM [96,512] -> [96,1] which is DMA'd out; the tiny
  normalize / L1 / mean finish runs on host.

  Layout: channels PAIR-STACKED on the partition axis as in v1: a
  [128, 4096] tile holds channel A on partitions 0..63, B on 64..127.

  Predicted engine busy per core: DVE ~118us (bound), PE ~83us,
  ScalarE ~67us, vs v1's DVE ~225us.
"""

from contextlib import ExitStack
import math

import numpy as np

import concourse.bass as bass
import concourse.tile as tile
from concourse import bacc, mybir
from concourse.bass_utils import run_bass_kernel_spmd

N_CORES = 8
C = 3
H = 512
W = 512
HW = H * W          # 262144
P = 128
HP = 64             # partitions per channel in a stacked pair
FP = HW // HP       # 4096
NB = 16
NPAIR = 3           # (t1,t2), (p2,t0), (p0,p1)
F32 = mybir.dt.float32
F16 = mybir.dt.float16

MASK_ON_GPS = False           # brightness sum + threshold on GPSIMD
XM_ON_GPS = False             # xm = x + off2 on GPSIMD
BETA = 128.0 / 225.0          # exp(-128(x-k/15)^2) = exp(-BETA (15x-k)^2)
A = 256.0 / 15.0              # d/dx of the up-ratio exponent
RS = 6.0                      # r-tile rescale (fp16 range)
SC = 15.0 * math.log(2.0)     # chain-start scale 2^15
CHAIN_STARTS = (0, 4, 8, 12)  # ascending-only chains of 4 bins each
CHAIN_STEPS = 3
MM_CHUNK = 512                # matmul moving free-dim (PSUM bank = 512 f32)
STATS_ROWS = 2 * NPAIR * NB   # 96


def _kernel_body(ctx, tc, stats_d, pred_d, target_d, repeat=1):
    nc = tc.nc
    stacks = ctx.enter_context(tc.tile_pool(name="stacks", bufs=1))
    maskp = ctx.enter_context(tc.tile_pool(name="maskp", bufs=1))
    vpool = ctx.enter_context(tc.tile_pool(name="vpool", bufs=2))
    epool = ctx.enter_context(tc.tile_pool(name="epool", bufs=9))
    ppool = ctx.enter_context(tc.tile_pool(name="ppool", bufs=1, space="PSUM"))
    spool = ctx.enter_context(tc.tile_pool(name="spool", bufs=1))
    scr = ctx.enter_context(tc.tile_pool(name="scr", bufs=1))
    pools = (stacks, maskp, scr, vpool, epool, ppool, spool)

    # Per-bin [128, 32] fp16 stationaries: bin k has ones at
    # (partitions 0..63, col 2k) and (64..127, col 2k+1), zeros elsewhere.
    # Matmul out [32, 512] lands at PSUM base 32*pair (base must be
    # 0/32/64); the zero columns contribute 0 to the other bins' rows.
    ones_k = []
    for k in range(NB):
        o = spool.tile([P, 2 * NB], F16, tag=f"ones{k}")
        nc.gpsimd.memset(o[:], 0.0)
        nc.gpsimd.memset(o[:HP, 2 * k : 2 * k + 1], 1.0)
        nc.gpsimd.memset(o[HP:, 2 * k + 1 : 2 * k + 2], 1.0)
        ones_k.append(o)

    # ACT bias constants must be APs: one [128, 1] column per value.
    bias_vals = [-BETA - RS, SC] + [-s0 / 15.0 for s0 in CHAIN_STARTS]
    biases = spool.tile([P, len(bias_vals)], F32, tag="biases")
    bias_ap = {}
    for i, v in enumerate(bias_vals):
        nc.gpsimd.memset(biases[:, i : i + 1], v)
        bias_ap[v] = biases[:, i : i + 1]

    for _ in range(repeat):
        _emit_pass(ctx, tc, pools, ones_k, bias_ap, stats_d, pred_d, target_d)


def _emit_pass(ctx, tc, pools, ones_k, bias_ap, stats_d, pred_d, target_d):
    nc = tc.nc
    add = mybir.AluOpType.add
    mult = mybir.AluOpType.mult
    is_le = mybir.AluOpType.is_le
    stacks, maskp, scr, vpool, epool, ppool, spool = pools

    def chan_ap(dram, c):
        return dram[c].rearrange("(q g) -> q g", q=HP)

    pair_srcs = [
        (chan_ap(target_d, 1), chan_ap(target_d, 2)),
        (chan_ap(pred_d, 2), chan_ap(target_d, 0)),
        (chan_ap(pred_d, 0), chan_ap(pred_d, 1)),
    ]
    # pair0 (mask inputs) first, plus base-0 re-reads of t0/t2 so the
    # brightness sum has all three channels at the same base partition.
    pair_tiles = []
    t = stacks.tile([P, FP], F32, tag="pair0")
    nc.sync.dma_start(out=t[:HP, :], in_=pair_srcs[0][0])
    nc.sync.dma_start(out=t[HP:, :], in_=pair_srcs[0][1])
    pair_tiles.append(t)
    m0 = scr.tile([P, FP], F32, tag="m0")
    nc.sync.dma_start(out=m0[:HP, :], in_=chan_ap(target_d, 0))
    m2 = scr.tile([P, FP], F32, tag="m2")
    nc.sync.dma_start(out=m2[:HP, :], in_=chan_ap(target_d, 2))
    for i, (a_ap, b_ap) in enumerate(pair_srcs[1:], start=1):
        t = stacks.tile([P, FP], F32, tag=f"pair{i}")
        nc.sync.dma_start(out=t[:HP, :], in_=a_ap)
        nc.sync.dma_start(out=t[HP:, :], in_=b_ap)
        pair_tiles.append(t)

    # off2 = 100 where masked out (brightness sum <= 1.2), else 0.
    # (walrus rejects 2-SBUF-input ops with mismatched base partitions, so
    # t0/t2 are re-read at base 0 rather than borrowed from the pair tiles)
    off2 = maskp.tile([P, FP], F32, tag="off2")
    s = off2[:HP, :]
    meng = nc.gpsimd if MASK_ON_GPS else nc.vector
    meng.tensor_tensor(out=s, in0=m0[:HP, :], in1=pair_tiles[0][:HP, :], op=add)
    meng.tensor_tensor(out=s, in0=s, in1=m2[:HP, :], op=add)
    meng.tensor_scalar(
        out=s, in0=s, scalar1=1.2, scalar2=100.0, op0=is_le, op1=mult
    )
    nc.sync.dma_start(out=off2[HP:, :], in_=s)  # replicate to upper half

    psum = ppool.tile([STATS_ROWS, MM_CHUNK], F32, tag="psum")
    nchunk = FP // MM_CHUNK
    # per pair-region: 16 bins x nchunk accumulating matmuls; first resets,
    # last closes the group.
    mm_total = NB * nchunk
    mm_count = [0, 0, 0]

    def mm(w, pi, k):
        base = 32 * pi
        for c in range(nchunk):
            n = mm_count[pi]
            nc.tensor.matmul(
                out=psum[base : base + 2 * NB, :],
                lhsT=ones_k[k][:],
                rhs=w[:, c * MM_CHUNK : (c + 1) * MM_CHUNK],
                start=(n == 0),
                stop=(n == mm_total - 1),
            )
            mm_count[pi] = n + 1

    # Per pair: xm + r_up prep, then the 4 ascending chains interleaved
    # step-by-step (4 independent stt sequences) so consecutive DVE ops are
    # never data-dependent (hides the DVE pipe DRAIN).  Pairs stay
    # sequential so each pair's PSUM accumulation group closes before the
    # next opens.
    for pi, x in enumerate(pair_tiles):
        xm = vpool.tile([P, FP], F16, tag="xm")
        xeng = nc.gpsimd if XM_ON_GPS else nc.vector
        xeng.tensor_tensor(out=xm[:], in0=x[:], in1=off2[:], op=add)
        r_up = vpool.tile([P, FP], F16, tag="r_up")
        nc.scalar.activation(
            out=r_up[:], in_=x[:], func=mybir.ActivationFunctionType.Exp,
            bias=bias_ap[-BETA - RS], scale=A,
        )
        wp = []
        for ci, s0 in enumerate(CHAIN_STARTS):
            sq = epool.tile([P, FP], F16, tag="e")
            nc.scalar.activation(
                out=sq[:], in_=xm[:], func=mybir.ActivationFunctionType.Square,
                bias=bias_ap[-s0 / 15.0], scale=1.0,
            )
            w0 = epool.tile([P, FP], F16, tag="e")
            nc.scalar.activation(
                out=w0[:], in_=sq[:], func=mybir.ActivationFunctionType.Exp,
                bias=bias_ap[SC], scale=-128.0,
            )
            mm(w0, pi, s0)
            wp.append(w0)
        for step in range(CHAIN_STEPS):
            for ci, s0 in enumerate(CHAIN_STARTS):
                k = s0 + step
                wn = epool.tile([P, FP], F16, tag="e")
                nc.vector.scalar_tensor_tensor(
                    out=wn[:], in0=r_up[:],
                    scalar=math.exp(-2.0 * BETA * k + RS),
                    in1=wp[ci][:], op0=mult, op1=mult,
                )
                mm(wn, pi, k + 1)
                wp[ci] = wn

    stats_t = spool.tile([STATS_ROWS, 1], F32, tag="stats")
    nc.vector.tensor_reduce(
        out=stats_t[:], in_=psum[:], axis=mybir.AxisListType.X,
        op=mybir.AluOpType.add,
    )
    nc.sync.dma_start(out=stats_d[:], in_=stats_t[:])


def build_nc(repeat=1):
    nc = bacc.Bacc(
        "TRN2", target_bir_lowering=False, debug=False, num_devices=N_CORES
    )
    pred = nc.dram_tensor("pred", [C, HW], F32, kind="ExternalInput").ap()
    target = nc.dram_tensor("target", [C, HW], F32, kind="ExternalInput").ap()
    stats = nc.dram_tensor(
        "stats", [STATS_ROWS, 1], F32, kind="ExternalOutput"
    ).ap()
    with tile.TileContext(nc) as tc:
        with ExitStack() as ctx:
            _kernel_body(ctx, tc, stats, pred, target, repeat=repeat)
    nc.compile()
    return nc


_NC_CACHE = {}


def _get_nc():
    if "nc" not in _NC_CACHE:
        _NC_CACHE["nc"] = build_nc()
    return _NC_CACHE["nc"]


# stats row -> (which hist 0=pred/1=target, channel): row = 32*pair + 2*bin + half
_PAIR_CHANNELS = [((1, 1), (1, 2)), ((0, 2), (1, 0)), ((0, 0), (0, 1))]


def stats_to_hists(stats):
    """[96, 1] per-core sums -> hist [2, C, NB] (pred, target) f64."""
    v = stats.reshape(NPAIR, NB, 2).astype(np.float64)
    hist = np.empty((2, C, NB), np.float64)
    for p in range(NPAIR):
        for half in range(2):
            which, ch = _PAIR_CHANNELS[p][half]
            hist[which, ch] = v[p, :, half]
    return hist


def finish_on_host(stats_list):
    diffs = []
    for stats in stats_list:
        hist = stats_to_hists(stats)
        hist_n = hist / (hist.sum(axis=-1, keepdims=True) + 1e-7)
        diffs.append(np.abs(hist_n[0] - hist_n[1]))
    return np.array(np.mean(np.stack(diffs)), dtype=np.float32)


def run(pred, target, **spmd_kwargs):
    nc = _get_nc()
    pred = np.ascontiguousarray(np.asarray(pred, dtype=np.float32))
    target = np.ascontiguousarray(np.asarray(target, dtype=np.float32))
    assert pred.shape == (N_CORES, C, H, W), pred.shape
    in_maps = [
        {
            "pred": pred[b].reshape(C, HW),
            "target": target[b].reshape(C, HW),
        }
        for b in range(N_CORES)
    ]
    res = run_bass_kernel_spmd(
        nc, in_maps, core_ids=list(range(N_CORES)), **spmd_kwargs
    )
    loss = finish_on_host([res.results[b]["stats"] for b in range(N_CORES)])
    return loss, res


def kernel(pred, target):
    loss, _ = run(pred, target)
    return loss
